# revision 1
# baseline (speedup 1.0000x reference)
"""ChannelMerger kernel for 8x Trainium2 NeuronCores (Bass/Tile).

Computes, for eeg [B,T,C], positions [B,C,2], heads [O,D]:
    emb     = fourier_emb(positions)              # [B,C,D], D = 2*12*12
    scores  = einsum('bcd,od->boc', emb, heads)   # [B,O,C]
    weights = softmax(scores, axis=2)
    out     = einsum('bct,boc->bot', eeg_ct, weights).transpose -> [B,T,O]

Sharding: data-parallel over batch B=32 -> 4 batches per core on 8 cores.
All compute (fourier, matmuls, softmax, weighted sum) runs on-device; the
host only shards/reshapes inputs and pads constants.
"""

import numpy as np

import concourse.bacc as bacc
import concourse.mybir as mybir
import concourse.tile as tile

# ---------------------------------------------------------------- constants
B, T, C = 32, 8192, 128
O = 64
N_FREQS = 12
N_IJ = N_FREQS * N_FREQS          # 144
D = 2 * N_IJ                      # 288
MARGIN = 0.2
N_CORES = 8
BPC = B // N_CORES                # batches per core = 4
TGROUP = 512                      # t rows per group
NGROUP = T // TGROUP              # 16
JI = 4                            # row interleave within a group
F32 = mybir.dt.float32


# ------------------------------------------------------------ host constants
def _host_constants(heads: np.ndarray):
    """Pure layout/padding transforms of `heads` + static tables."""
    width = 1.0 + 2.0 * MARGIN
    # Frequencies in TURNS (cycles): loc_rad = 2*pi * (pos_x*p_i + pos_y*p_j).
    # Working in turns lets the device reduce the phase into [-pi, pi] with a
    # round-to-nearest int cast before the Sin table lookup.
    p = np.arange(N_FREQS, dtype=np.float64) / width

    # Per-partition frequency columns for the transposed loc computation.
    # Chunk c covers ij = 128c + k (k = partition); entries past 143 are 0
    # and their heads rows are zero-padded, so they contribute nothing.
    pij = np.zeros((128, 4), dtype=np.float32)
    for c in range(2):
        for k in range(128):
            ij = 128 * c + k
            if ij < N_IJ:
                pij[k, 2 * c + 0] = p[ij // N_FREQS]
                pij[k, 2 * c + 1] = p[ij % N_FREQS]

    # headsT chunks [K=128, O] for the 4 embT chunks (cos0, cos1, sin0, sin1)
    ht4 = np.zeros((128, 4 * O), dtype=np.float32)
    ht4[:, 0 * O:1 * O] = heads[:, 0:128].T               # cos ij 0..127
    ht4[:16, 1 * O:2 * O] = heads[:, 128:144].T           # cos ij 128..143
    ht4[:, 2 * O:3 * O] = heads[:, 144:272].T             # sin ij 0..127
    ht4[:16, 3 * O:4 * O] = heads[:, 272:288].T           # sin ij 128..143

    ident = np.eye(128, dtype=np.float32)
    return pij, ht4, ident


def _pos_broadcast(positions_core: np.ndarray) -> np.ndarray:
    """[BPC,C,2] -> [128, BPC*256] with (x+MARGIN | y+MARGIN) per batch,
    replicated across all 128 partitions (layout-only transform)."""
    pos = positions_core.astype(np.float32) + np.float32(MARGIN)
    row = np.concatenate(
        [np.concatenate([pos[b, :, 0], pos[b, :, 1]]) for b in range(BPC)]
    )  # [BPC*256]
    return np.broadcast_to(row, (128, row.size)).copy()


# ------------------------------------------------------------- device kernel
def _build_nc(debug=False):
    # Bacc (not plain Bass): finalize() runs generate_event_semaphores,
    # which splits multi-sem waits (TRN2 allows 1 wait per instruction).
    nc = bacc.Bacc()
    eeg = nc.declare_dram_parameter("eeg", [BPC, T, C], F32, isOutput=False)
    posb = nc.declare_dram_parameter("posb", [128, BPC * 2 * C], F32, isOutput=False)
    ht4 = nc.declare_dram_parameter("ht4", [128, 4 * O], F32, isOutput=False)
    pij = nc.declare_dram_parameter("pij", [128, 4], F32, isOutput=False)
    identity = nc.declare_dram_parameter("identity", [128, 128], F32, isOutput=False)
    out = nc.declare_dram_parameter("out", [BPC, T, O], F32, isOutput=True)
    if debug:
        wt_out = nc.declare_dram_parameter("wt_out", [128, BPC * O], F32, isOutput=True)
        emb_out = nc.declare_dram_parameter("emb_out", [128, 4 * 128], F32, isOutput=True)

    TWO_PI = float(2.0 * np.pi)
    I32 = mybir.dt.int32
    BF16 = mybir.dt.bfloat16

    with tile.TileContext(nc) as tc:
        with tc.tile_pool(name="consts", bufs=1) as cpool:
            # PE warm-up: the HAM clock gate keeps the PE at 1.2 GHz until it
            # sees ~3.4us of sustained matmul activity. Burn a burst of cheap
            # bf16 matmuls while the initial DMAs land so the real work runs
            # at 2.4 GHz from the start.
            wu_a = cpool.tile([128, 128], BF16)
            wu_b = cpool.tile([128, 512], BF16)
            nc.vector.memset(wu_a, 1.0)
            nc.vector.memset(wu_b, 1.0)
            with tc.tile_pool(name="wups", bufs=1, space="PSUM") as wups:
                wu_ps = wups.tile([128, 512], F32)
                for _ in range(24):
                    nc.tensor.matmul(out=wu_ps, lhsT=wu_a, rhs=wu_b,
                                     start=True, stop=True)
            posb_sb = cpool.tile([128, BPC * 2 * C], F32)
            nc.sync.dma_start(out=posb_sb, in_=posb[:, :])
            pij_sb = cpool.tile([128, 4], F32)
            nc.sync.dma_start(out=pij_sb, in_=pij[:, :])
            ht4_sb = cpool.tile([128, 4 * O], F32)
            nc.sync.dma_start(out=ht4_sb, in_=ht4[:, :])
            ident_sb = cpool.tile([128, 128], F32)
            nc.sync.dma_start(out=ident_sb, in_=identity[:, :])
            # softmaxed channel weights, transposed: [C, O] per batch
            wt_all = cpool.tile([128, BPC * O], F32)

            with (
                tc.tile_pool(name="ein", bufs=2) as ein,
                tc.tile_pool(name="wsb", bufs=1) as wsb,
                tc.tile_pool(name="ets", bufs=16) as ets,
                tc.tile_pool(name="osb", bufs=2) as osb,
                tc.tile_pool(name="wps", bufs=1, space="PSUM") as wps,
                tc.tile_pool(name="etp", bufs=4, space="PSUM") as etp,
                tc.tile_pool(name="otp", bufs=2, space="PSUM") as otp,
            ):

                # Kick off all eeg loads first: one bulk 4 MB DMA per batch,
                # double-buffered (b>=2 waits for slot release), overlapping
                # the weights computation below.
                e_tiles = []
                for b in range(BPC):
                    e_sb = ein.tile([128, NGROUP * TGROUP], F32, tag="e", name=f"e_{b}")
                    eeg_r = eeg[b].rearrange("(g p j) c -> p g (j c)", p=128, j=JI)
                    nc.sync.dma_start(
                        out=e_sb.rearrange("p (g x) -> p g x", g=NGROUP), in_=eeg_r
                    )
                    e_tiles.append(e_sb)

                # ---------- phase 0: fourier emb + scores + softmax --------
                # All 4 batches processed in single wide ops where possible.
                pv = posb_sb.rearrange("p (b s c) -> p b s c", b=BPC, s=2)
                x_all = pv[:, :, 0, :]   # [128, BPC, C]
                y_all = pv[:, :, 1, :]
                embq = wsb.tile([128, BPC, 4, 128], F32, tag="embq")
                for c in range(2):
                    # phase in turns: t = x*p_i + y*p_j  (>= 0, < ~19)
                    t1 = wsb.tile([128, BPC, 128], F32, tag="t1")
                    tt = wsb.tile([128, BPC, 128], F32, tag="tt")
                    nc.vector.tensor_scalar_mul(
                        out=t1, in0=x_all, scalar1=pij_sb[:, 2 * c:2 * c + 1]
                    )
                    nc.vector.tensor_scalar_mul(
                        out=tt, in0=y_all, scalar1=pij_sb[:, 2 * c + 1:2 * c + 2]
                    )
                    nc.vector.tensor_add(out=tt, in0=tt, in1=t1)
                    tc4 = wsb.tile([128, BPC, 128], F32, tag="tc4")
                    nc.vector.tensor_scalar_add(out=tc4, in0=tt, scalar1=0.25)
                    # cos chunk (t+0.25) -> q=c, sin chunk -> q=2+c.
                    # Reduce phase via round-to-nearest-even f32->i32 cast:
                    # r = t - rne(t) in [-0.5, 0.5]; sin(2pi*t) = Sin(2pi*r).
                    for src_t, q in ((tc4, c), (tt, 2 + c)):
                        ki = wsb.tile([128, BPC, 128], I32, tag="ki")
                        kf = wsb.tile([128, BPC, 128], F32, tag="kf")
                        nc.vector.tensor_copy(out=ki, in_=src_t)
                        nc.vector.tensor_copy(out=kf, in_=ki)
                        rr = wsb.tile([128, BPC, 128], F32, tag="rr")
                        nc.vector.tensor_sub(out=rr, in0=src_t, in1=kf)
                        nc.scalar.activation(
                            out=embq[:, :, q, :], in_=rr,
                            func=mybir.ActivationFunctionType.Sin,
                            scale=TWO_PI, bias=0.0,
                        )
                if debug:
                    nc.sync.dma_start(
                        out=emb_out[:, :],
                        in_=embq[:, 0, :, :],
                    )
                scores_ps = wps.tile([O, BPC, 128], F32, tag="scores")
                for b in range(BPC):
                    for q in range(4):
                        nc.tensor.matmul(
                            out=scores_ps[:, b, :],
                            lhsT=ht4_sb[:, q * O:(q + 1) * O],
                            rhs=embq[:, b, q, :],
                            start=(q == 0), stop=(q == 3),
                        )
                # scores are bounded (|s| < ~10): plain exp is fp32-safe and
                # softmax is shift-invariant, so skip the max-subtraction —
                # one less DVE hop on the critical path to the weights.
                probs = wsb.tile([O, BPC, 128], F32, tag="probs")
                ssum = wsb.tile([O, BPC], F32, tag="ssum")
                for b in range(BPC):
                    nc.scalar.activation(
                        out=probs[:, b, :], in_=scores_ps[:, b, :],
                        func=mybir.ActivationFunctionType.Exp,
                        bias=0.0, accum_out=ssum[:, b:b + 1],
                    )
                # Prologue: batch-0's first two groups of eeg transposes
                # (PSUM-bank-limited to 2 groups) keep the PE busy while the
                # softmax chain below resolves, instead of stalling the
                # in-order PE queue at the weight transposes.
                NPRO = 2
                pro_ets = {}
                for g in range(NPRO):
                    eg = e_tiles[0][:, g * TGROUP:(g + 1) * TGROUP]
                    pair = []
                    for h in range(2):
                        et_ps = etp.tile([128, 256], F32, tag="etps",
                                         name=f"pro_etps_{g}_{h}")
                        for jj in range(2):
                            j = 2 * h + jj
                            nc.tensor.transpose(
                                out=et_ps[:, jj * 128:(jj + 1) * 128],
                                in_=eg[:, j * 128:(j + 1) * 128],
                                identity=ident_sb,
                            )
                        et_sb = ets.tile([128, 256], F32, tag="etsb",
                                         name=f"pro_etsb_{g}_{h}")
                        nc.vector.tensor_copy(out=et_sb, in_=et_ps)
                        pair.append(et_sb)
                    pro_ets[g] = pair

                rcp = wsb.tile([O, BPC], F32, tag="rcp")
                nc.vector.reciprocal(out=rcp, in_=ssum)
                wgt = wsb.tile([O, BPC, 128], F32, tag="wgt")
                wt_ps = wps.tile([128, BPC, O], F32, tag="wtps")
                for b in range(BPC):
                    nc.vector.tensor_scalar_mul(
                        out=wgt[:, b, :], in0=probs[:, b, :],
                        scalar1=rcp[:, b:b + 1],
                    )
                    nc.tensor.transpose(
                        out=wt_ps[:, b, :], in_=wgt[:, b, :],
                        identity=ident_sb[0:O, 0:O],
                    )
                nc.vector.tensor_copy(out=wt_all, in_=wt_ps)
                if debug:
                    nc.sync.dma_start(out=wt_out[:, :], in_=wt_all)

                # ---------- main loop: out[t,o] = sum_c eeg[t,c]*w[o,c] ----
                for b in range(BPC):
                    out_r = out[b].rearrange("(g p j) o -> p g (j o)", p=128, j=JI)
                    wt_b = wt_all[:, b * O:(b + 1) * O]
                    e_sb = e_tiles[b]
                    o_sb = osb.tile([128, NGROUP * JI * O], F32, tag="osb")
                    for g in range(NGROUP):
                        eg = e_sb[:, g * TGROUP:(g + 1) * TGROUP]
                        out_ps = otp.tile([128, JI * O], F32, tag="outps")
                        for h in range(2):  # transpose pairs
                            if b == 0 and g < NPRO:
                                et_sb = pro_ets[g][h]  # transposed in prologue
                            else:
                                et_ps = etp.tile([128, 256], F32, tag="etps")
                                for jj in range(2):
                                    j = 2 * h + jj
                                    nc.tensor.transpose(
                                        out=et_ps[:, jj * 128:(jj + 1) * 128],
                                        in_=eg[:, j * 128:(j + 1) * 128],
                                        identity=ident_sb,
                                    )
                                et_sb = ets.tile([128, 256], F32, tag="etsb")
                                nc.vector.tensor_copy(out=et_sb, in_=et_ps)
                            for jj in range(2):
                                j = 2 * h + jj
                                nc.tensor.matmul(
                                    out=out_ps[:, j * O:(j + 1) * O],
                                    lhsT=et_sb[:, jj * 128:(jj + 1) * 128],
                                    rhs=wt_b,
                                    start=True, stop=True,
                                )
                        nc.scalar.copy(
                            out=o_sb[:, g * JI * O:(g + 1) * JI * O], in_=out_ps
                        )
                        nsp = 4 if b == BPC - 1 else 2  # finer drain at the tail
                        gper = NGROUP // nsp
                        if g % gper == gper - 1:
                            part = g // gper
                            hw = gper * JI * O
                            nc.sync.dma_start(
                                out=out_r[:, part * gper:(part + 1) * gper, :],
                                in_=o_sb[:, part * hw:(part + 1) * hw].rearrange(
                                    "p (g x) -> p g x", g=gper
                                ),
                            )
    nc.finalize()
    return nc


_NC_CACHE = None


def _get_nc():
    global _NC_CACHE
    if _NC_CACHE is None:
        _NC_CACHE = _build_nc()
    return _NC_CACHE


def _make_in_maps(eeg, positions, heads):
    pij, ht4, ident = _host_constants(np.asarray(heads, dtype=np.float32))
    eeg = np.asarray(eeg, dtype=np.float32)
    positions = np.asarray(positions, dtype=np.float32)
    in_maps = []
    for core in range(N_CORES):
        sl = slice(core * BPC, (core + 1) * BPC)
        in_maps.append({
            "eeg": np.ascontiguousarray(eeg[sl]),
            "posb": _pos_broadcast(positions[sl]),
            "ht4": ht4,
            "pij": pij,
            "identity": ident,
        })
    return in_maps


def kernel(eeg, positions, heads, sub=None, **_unused):
    from concourse.bass_utils import run_bass_kernel_spmd

    nc = _get_nc()
    in_maps = _make_in_maps(eeg, positions, heads)
    res = run_bass_kernel_spmd(nc, in_maps, list(range(N_CORES)))
    out = np.concatenate([res.results[c]["out"] for c in range(N_CORES)], axis=0)
    return out



# revision 2
# speedup vs baseline: 2.1988x; 2.1988x over previous
"""ChannelMerger kernel for 8x Trainium2 NeuronCores (Bass/Tile).

Computes, for eeg [B,T,C], positions [B,C,2], heads [O,D]:
    emb     = fourier_emb(positions)              # [B,C,D], D = 2*12*12
    scores  = einsum('bcd,od->boc', emb, heads)   # [B,O,C]
    weights = softmax(scores, axis=2)
    out     = einsum('bct,boc->bot', eeg_ct, weights).transpose -> [B,T,O]

Sharding: data-parallel over batch B=32 -> 4 batches per core on 8 cores.

Layout strategy: the host pre-transposes eeg to [B, C, T] and casts to
fp16, so the device streams it straight through the PE as the moving
matmul operand with the (tiny, softmaxed) channel weights stationary --
no on-device transposes at all.  The result is produced as [B, O, T]
fp16 (contiguous DMA) and the host casts/transposes it back to
[B, T, O] f32.  fp16 halves HBM traffic in both directions; with f32
accumulation in PSUM the end-to-end relative error stays ~1e-3.
"""

import numpy as np

import concourse.bacc as bacc
import concourse.mybir as mybir
import concourse.tile as tile

# ---------------------------------------------------------------- constants
B, T, C = 32, 8192, 128
O = 64
N_FREQS = 12
N_IJ = N_FREQS * N_FREQS          # 144
D = 2 * N_IJ                      # 288
MARGIN = 0.2
N_CORES = 8
BPC = B // N_CORES                # batches per core = 4
TCH = 512                         # t columns per matmul chunk (1 PSUM bank)
NCH = T // TCH                    # 16
QCH = 4                           # chunks per output DMA (2048 t cols)
F32 = mybir.dt.float32
F16 = mybir.dt.float16


# ------------------------------------------------------------ host constants
def _host_constants(heads: np.ndarray):
    """Pure layout/padding transforms of `heads` + static tables."""
    width = 1.0 + 2.0 * MARGIN
    # Frequencies in TURNS (cycles): loc_rad = 2*pi * (pos_x*p_i + pos_y*p_j).
    # Working in turns lets the device reduce the phase into [-pi, pi] with a
    # round-to-nearest int cast before the Sin table lookup.
    p = np.arange(N_FREQS, dtype=np.float64) / width

    # Per-partition frequency columns for the transposed loc computation.
    # Chunk c covers ij = 128c + k (k = partition); entries past 143 are 0
    # and their heads rows are zero-padded, so they contribute nothing.
    pij = np.zeros((128, 4), dtype=np.float32)
    for c in range(2):
        for k in range(128):
            ij = 128 * c + k
            if ij < N_IJ:
                pij[k, 2 * c + 0] = p[ij // N_FREQS]
                pij[k, 2 * c + 1] = p[ij % N_FREQS]

    # headsT chunks [K=128, O] for the 4 embT chunks (cos0, cos1, sin0, sin1)
    ht4 = np.zeros((128, 4 * O), dtype=np.float32)
    ht4[:, 0 * O:1 * O] = heads[:, 0:128].T               # cos ij 0..127
    ht4[:16, 1 * O:2 * O] = heads[:, 128:144].T           # cos ij 128..143
    ht4[:, 2 * O:3 * O] = heads[:, 144:272].T             # sin ij 0..127
    ht4[:16, 3 * O:4 * O] = heads[:, 272:288].T           # sin ij 128..143

    ident = np.eye(128, dtype=np.float32)
    return pij, ht4, ident


def _pos_broadcast(positions_core: np.ndarray) -> np.ndarray:
    """[BPC,C,2] -> [128, BPC*256] with (x+MARGIN | y+MARGIN) per batch,
    replicated across all 128 partitions (layout-only transform)."""
    pos = positions_core.astype(np.float32) + np.float32(MARGIN)
    row = np.concatenate(
        [np.concatenate([pos[b, :, 0], pos[b, :, 1]]) for b in range(BPC)]
    )  # [BPC*256]
    return np.broadcast_to(row, (128, row.size)).copy()


# ------------------------------------------------------------- device kernel
def _build_nc():
    # Bacc (not plain Bass): finalize() runs generate_event_semaphores,
    # which splits multi-sem waits (TRN2 allows 1 wait per instruction).
    nc = bacc.Bacc()
    eegt = nc.declare_dram_parameter("eegt", [BPC, C, T], F16, isOutput=False)
    posb = nc.declare_dram_parameter("posb", [128, BPC * 2 * C], F32, isOutput=False)
    ht4 = nc.declare_dram_parameter("ht4", [128, 4 * O], F32, isOutput=False)
    pij = nc.declare_dram_parameter("pij", [128, 4], F32, isOutput=False)
    identity = nc.declare_dram_parameter("identity", [128, 128], F32, isOutput=False)
    outT = nc.declare_dram_parameter("outT", [BPC, O, T], F16, isOutput=True)

    TWO_PI = float(2.0 * np.pi)
    I32 = mybir.dt.int32

    with tile.TileContext(nc) as tc:
        with tc.tile_pool(name="consts", bufs=1) as cpool:
            # Small weight-phase constants first: they gate the softmax
            # weights, which gate every matmul.  The bulk eeg loads queue
            # right behind and stream while the weights phase computes.
            posb_sb = cpool.tile([128, BPC * 2 * C], F32)
            nc.sync.dma_start(out=posb_sb, in_=posb[:, :])
            pij_sb = cpool.tile([128, 4], F32)
            nc.sync.dma_start(out=pij_sb, in_=pij[:, :])
            ht4_sb = cpool.tile([128, 4 * O], F32)
            nc.sync.dma_start(out=ht4_sb, in_=ht4[:, :])
            ident_sb = cpool.tile([128, 128], F32)
            nc.sync.dma_start(out=ident_sb, in_=identity[:, :])

            # Bulk eeg loads: one 2 MB DMA per batch, all resident in SBUF
            # (4 x 16 KB/partition).
            e_tiles = []
            for b in range(BPC):
                e_sb = cpool.tile([C, T], F16, name=f"e_{b}")
                nc.sync.dma_start(out=e_sb, in_=eegt[b])
                e_tiles.append(e_sb)

            # fp16 softmaxed channel weights, transposed: [C, O] per batch
            wt16 = cpool.tile([128, BPC * O], F16)

            with (
                tc.tile_pool(name="wsb", bufs=1) as wsb,
                tc.tile_pool(name="osb", bufs=2) as osb,
                tc.tile_pool(name="wps", bufs=1, space="PSUM") as wps,
                tc.tile_pool(name="otp", bufs=4, space="PSUM") as otp,
            ):
                # ---------- phase 0: fourier emb + scores + softmax --------
                # All 4 batches processed in single wide ops where possible.
                pv = posb_sb.rearrange("p (b s c) -> p b s c", b=BPC, s=2)
                x_all = pv[:, :, 0, :]   # [128, BPC, C]
                y_all = pv[:, :, 1, :]
                embq = wsb.tile([128, BPC, 4, 128], F32, tag="embq")
                for c in range(2):
                    # phase in turns: t = x*p_i + y*p_j  (>= 0, < ~19)
                    t1 = wsb.tile([128, BPC, 128], F32, tag="t1")
                    tt = wsb.tile([128, BPC, 128], F32, tag="tt")
                    nc.vector.tensor_scalar_mul(
                        out=t1, in0=x_all, scalar1=pij_sb[:, 2 * c:2 * c + 1]
                    )
                    nc.vector.tensor_scalar_mul(
                        out=tt, in0=y_all, scalar1=pij_sb[:, 2 * c + 1:2 * c + 2]
                    )
                    nc.vector.tensor_add(out=tt, in0=tt, in1=t1)
                    tc4 = wsb.tile([128, BPC, 128], F32, tag="tc4")
                    nc.vector.tensor_scalar_add(out=tc4, in0=tt, scalar1=0.25)
                    # cos chunk (t+0.25) -> q=c, sin chunk -> q=2+c.
                    # Reduce phase via round-to-nearest-even f32->i32 cast:
                    # r = t - rne(t) in [-0.5, 0.5]; sin(2pi*t) = Sin(2pi*r).
                    for src_t, q in ((tc4, c), (tt, 2 + c)):
                        ki = wsb.tile([128, BPC, 128], I32, tag="ki")
                        kf = wsb.tile([128, BPC, 128], F32, tag="kf")
                        nc.vector.tensor_copy(out=ki, in_=src_t)
                        nc.vector.tensor_copy(out=kf, in_=ki)
                        rr = wsb.tile([128, BPC, 128], F32, tag="rr")
                        nc.vector.tensor_sub(out=rr, in0=src_t, in1=kf)
                        nc.scalar.activation(
                            out=embq[:, :, q, :], in_=rr,
                            func=mybir.ActivationFunctionType.Sin,
                            scale=TWO_PI, bias=0.0,
                        )
                scores_ps = wps.tile([O, BPC, 128], F32, tag="scores")
                for b in range(BPC):
                    for q in range(4):
                        nc.tensor.matmul(
                            out=scores_ps[:, b, :],
                            lhsT=ht4_sb[:, q * O:(q + 1) * O],
                            rhs=embq[:, b, q, :],
                            start=(q == 0), stop=(q == 3),
                        )
                # scores are bounded (|s| < ~10): plain exp is fp32-safe and
                # softmax is shift-invariant, so skip the max-subtraction —
                # one less DVE hop on the critical path to the weights.
                probs = wsb.tile([O, BPC, 128], F32, tag="probs")
                ssum = wsb.tile([O, BPC], F32, tag="ssum")
                for b in range(BPC):
                    nc.scalar.activation(
                        out=probs[:, b, :], in_=scores_ps[:, b, :],
                        func=mybir.ActivationFunctionType.Exp,
                        bias=0.0, accum_out=ssum[:, b:b + 1],
                    )
                rcp = wsb.tile([O, BPC], F32, tag="rcp")
                nc.vector.reciprocal(out=rcp, in_=ssum)
                wgt = wsb.tile([O, BPC, 128], F32, tag="wgt")
                wt_ps = wps.tile([128, BPC, O], F32, tag="wtps")
                for b in range(BPC):
                    nc.vector.tensor_scalar_mul(
                        out=wgt[:, b, :], in0=probs[:, b, :],
                        scalar1=rcp[:, b:b + 1],
                    )
                    nc.tensor.transpose(
                        out=wt_ps[:, b, :], in_=wgt[:, b, :],
                        identity=ident_sb[0:O, 0:O],
                    )
                # cast to fp16 for the streaming matmuls
                nc.vector.tensor_copy(
                    out=wt16.rearrange("p (b o) -> p b o", b=BPC), in_=wt_ps
                )

                # ---------- main loop: outT[o,t] = sum_c w[c,o]*eegT[c,t] --
                # Two batches share one 128-partition PSUM tile via column
                # tiling: batch 2p -> array cols 0-63 / PSUM partitions
                # 0-63, batch 2p+1 -> cols 64-127.  The paired matmuls run
                # concurrently on disjoint column groups, and the PSUM ->
                # SBUF cast-copy runs at full 128-lane width.
                for p in range(BPC // 2):
                    o_sb = osb.tile([128, T], F16, tag="osb")
                    for g in range(NCH):
                        ps = otp.tile([128, TCH], F32, tag="outps")
                        for h in range(2):
                            bidx = 2 * p + h
                            nc.tensor.matmul(
                                out=ps[64 * h:64 * (h + 1), :],
                                lhsT=wt16[:, bidx * O:(bidx + 1) * O],
                                rhs=e_tiles[bidx][:, g * TCH:(g + 1) * TCH],
                                start=True, stop=True,
                                tile_position=(0, 64 * h),
                            )
                        cp = nc.scalar.copy if g % 2 else nc.vector.tensor_copy
                        cp(out=o_sb[:, g * TCH:(g + 1) * TCH], in_=ps)
                        if g % QCH == QCH - 1:
                            q = g // QCH
                            hw = QCH * TCH
                            nc.sync.dma_start(
                                out=outT[2 * p:2 * p + 2, :,
                                         q * hw:(q + 1) * hw].rearrange(
                                    "b o t -> (b o) t"
                                ),
                                in_=o_sb[:, q * hw:(q + 1) * hw],
                            )
    nc.finalize()
    return nc


_NC_CACHE = None


def _get_nc():
    global _NC_CACHE
    if _NC_CACHE is None:
        _NC_CACHE = _build_nc()
    return _NC_CACHE


def _make_in_maps(eeg, positions, heads):
    pij, ht4, ident = _host_constants(np.asarray(heads, dtype=np.float32))
    eeg16 = np.asarray(eeg, dtype=np.float16)           # [B, T, C]
    positions = np.asarray(positions, dtype=np.float32)
    in_maps = []
    for core in range(N_CORES):
        sl = slice(core * BPC, (core + 1) * BPC)
        in_maps.append({
            "eegt": np.ascontiguousarray(eeg16[sl].transpose(0, 2, 1)),
            "posb": _pos_broadcast(positions[sl]),
            "ht4": ht4,
            "pij": pij,
            "identity": ident,
        })
    return in_maps


def kernel(eeg, positions, heads, sub=None, **_unused):
    from concourse.bass_utils import run_bass_kernel_spmd

    nc = _get_nc()
    in_maps = _make_in_maps(eeg, positions, heads)
    res = run_bass_kernel_spmd(nc, in_maps, list(range(N_CORES)))
    outT = np.concatenate(
        [res.results[c]["outT"] for c in range(N_CORES)], axis=0
    )  # [B, O, T] fp16
    return outT.astype(np.float32).transpose(0, 2, 1)


# revision 3
# speedup vs baseline: 2.2303x; 1.0143x over previous
"""ChannelMerger kernel for 8x Trainium2 NeuronCores (Bass/Tile).

Computes, for eeg [B,T,C], positions [B,C,2], heads [O,D]:
    emb     = fourier_emb(positions)              # [B,C,D], D = 2*12*12
    scores  = einsum('bcd,od->boc', emb, heads)   # [B,O,C]
    weights = softmax(scores, axis=2)
    out     = einsum('bct,boc->bot', eeg_ct, weights).transpose -> [B,T,O]

Sharding: data-parallel over batch B=32 -> 4 batches per core on 8 cores.

Layout strategy: the host pre-transposes eeg to [B, C, T] and casts to
fp16, so the device streams it straight through the PE as the moving
matmul operand with the (tiny, softmaxed) channel weights stationary --
no on-device data transposes.  The result is produced as [B, O, T] fp16
(contiguous DMA) and the host casts/transposes it back to [B, T, O]
f32.  fp16 halves HBM traffic in both directions; with f32 softmax and
f32 PSUM accumulation the end-to-end relative error stays ~1e-3.

All math (fourier, scores, softmax, weighted sum) runs on-device; the
host only shards/reshapes/casts inputs and pads constants.
"""

import numpy as np

import concourse.bacc as bacc
import concourse.mybir as mybir
import concourse.tile as tile

# ---------------------------------------------------------------- constants
B, T, C = 32, 8192, 128
O = 64
N_FREQS = 12
N_IJ = N_FREQS * N_FREQS          # 144
D = 2 * N_IJ                      # 288
MARGIN = 0.2
N_CORES = 8
BPC = B // N_CORES                # batches per core = 4
TCH = 512                         # t columns per matmul chunk (1 PSUM bank)
NCH = T // TCH                    # 16
QCH = 4                           # chunks per output DMA (2048 t cols)
TH = T // 2                       # eeg half-load width
F32 = mybir.dt.float32
F16 = mybir.dt.float16


# ------------------------------------------------------------ host constants
def _host_constants(heads: np.ndarray):
    """Pure layout/padding transforms of `heads` + static tables."""
    width = 1.0 + 2.0 * MARGIN
    # Frequencies in TURNS (cycles): loc_rad = 2*pi * (pos_x*p_i + pos_y*p_j).
    # Working in turns lets the device reduce the phase into [-pi, pi] with a
    # round-to-nearest int cast before the Sin table lookup.
    p = np.arange(N_FREQS, dtype=np.float64) / width

    # Per-partition frequency columns for the transposed loc computation.
    # Chunk c covers ij = 128c + k (k = partition); entries past 143 are 0
    # and their heads rows are zero-padded, so they contribute nothing.
    pij = np.zeros((128, 4), dtype=np.float32)
    for c in range(2):
        for k in range(128):
            ij = 128 * c + k
            if ij < N_IJ:
                pij[k, 2 * c + 0] = p[ij // N_FREQS]
                pij[k, 2 * c + 1] = p[ij % N_FREQS]

    # headsT chunks [K=128, O] for the 4 embT chunks (cos0, cos1, sin0, sin1)
    ht4 = np.zeros((128, 4 * O), dtype=np.float32)
    ht4[:, 0 * O:1 * O] = heads[:, 0:128].T               # cos ij 0..127
    ht4[:16, 1 * O:2 * O] = heads[:, 128:144].T           # cos ij 128..143
    ht4[:, 2 * O:3 * O] = heads[:, 144:272].T             # sin ij 0..127
    ht4[:16, 3 * O:4 * O] = heads[:, 272:288].T           # sin ij 128..143

    ident = np.eye(128, dtype=np.float32)
    return pij, ht4.astype(np.float16), ident


def _pos_row(positions_core: np.ndarray) -> np.ndarray:
    """[BPC,C,2] -> [1, BPC*256] with (x+MARGIN | y+MARGIN) per batch.
    Single row; the device broadcasts it across partitions via the PE."""
    pos = positions_core.astype(np.float32) + np.float32(MARGIN)
    row = np.concatenate(
        [np.concatenate([pos[b, :, 0], pos[b, :, 1]]) for b in range(BPC)]
    )  # [BPC*256]
    return row.reshape(1, row.size).copy()


# ------------------------------------------------------------- device kernel
def _build_nc():
    # Bacc (not plain Bass): finalize() runs generate_event_semaphores,
    # which splits multi-sem waits (TRN2 allows 1 wait per instruction).
    nc = bacc.Bacc()
    eegt = nc.declare_dram_parameter("eegt", [BPC, C, T], F16, isOutput=False)
    posn = nc.declare_dram_parameter("posn", [1, BPC * 2 * C], F32, isOutput=False)
    ht4 = nc.declare_dram_parameter("ht4", [128, 4 * O], F16, isOutput=False)
    pij = nc.declare_dram_parameter("pij", [128, 4], F32, isOutput=False)
    identity = nc.declare_dram_parameter("identity", [128, 128], F32, isOutput=False)
    outT = nc.declare_dram_parameter("outT", [BPC, O, T], F16, isOutput=True)

    TWO_PI = float(2.0 * np.pi)
    I32 = mybir.dt.int32
    BF16 = mybir.dt.bfloat16

    with tile.TileContext(nc) as tc:
        with tc.tile_pool(name="consts", bufs=1) as cpool:
            # Small weight-phase constants first: they gate the softmax
            # weights, which gate every matmul.  The bulk eeg loads queue
            # right behind and stream while the weights phase computes.
            posn_sb = cpool.tile([1, BPC * 2 * C], F32)
            nc.sync.dma_start(out=posn_sb, in_=posn[:, :])
            pij_sb = cpool.tile([128, 4], F32)
            nc.sync.dma_start(out=pij_sb, in_=pij[:, :])
            ht4_sb = cpool.tile([128, 4 * O], F16)
            nc.sync.dma_start(out=ht4_sb, in_=ht4[:, :])
            ident_sb = cpool.tile([128, 128], F32)
            nc.sync.dma_start(out=ident_sb, in_=identity[:, :])

            # Bulk eeg loads, split in T-halves so pair-0 matmuls can start
            # as soon as the first halves of batches 0/1 land.
            e_tiles = [cpool.tile([C, T], F16, name=f"e_{b}") for b in range(BPC)]
            for h in range(2):
                for b in range(BPC):
                    sl = slice(h * TH, (h + 1) * TH)
                    nc.sync.dma_start(out=e_tiles[b][:, sl], in_=eegt[b][:, sl])

            # fp16 softmaxed channel weights, transposed: [C, O] per batch
            wt16 = cpool.tile([128, BPC * O], F16)
            ones_sb = cpool.tile([1, 128], F32)
            nc.vector.memset(ones_sb, 1.0)
            posb_sb = cpool.tile([128, BPC * 2 * C], F32)

            # Broadcast the position row to all 128 partitions via the PE
            # (ones[1,128].T @ posn[1,1024]); first thing in the PE queue.
            with tc.tile_pool(name="pbp", bufs=2, space="PSUM") as pbp:
                for half in range(2):
                    pb = pbp.tile([128, 512], F32, tag="pb")
                    nc.tensor.matmul(
                        out=pb, lhsT=ones_sb,
                        rhs=posn_sb[:, half * 512:(half + 1) * 512],
                        start=True, stop=True,
                    )
                    cp = nc.scalar.copy if half else nc.vector.tensor_copy
                    cp(out=posb_sb[:, half * 512:(half + 1) * 512], in_=pb)

            # PE warm-up: the HAM clock gate keeps the PE at 1.2 GHz until
            # it sees ~3.4us of sustained matmul activity.  Burn a burst of
            # cheap bf16 matmuls while the DMAs land so the scores and main
            # matmuls run at 2.4 GHz.
            wu_a = cpool.tile([128, 128], BF16)
            wu_b = cpool.tile([128, 512], BF16)
            nc.vector.memset(wu_a, 1.0)
            nc.vector.memset(wu_b, 1.0)
            with tc.tile_pool(name="wups", bufs=1, space="PSUM") as wups:
                wu_ps = wups.tile([128, 512], F32)
                for _ in range(12):
                    nc.tensor.matmul(out=wu_ps, lhsT=wu_a, rhs=wu_b,
                                     start=True, stop=True)

            with (
                tc.tile_pool(name="wsb", bufs=1) as wsb,
                tc.tile_pool(name="osb", bufs=2) as osb,
                tc.tile_pool(name="wps", bufs=1, space="PSUM") as wps,
                tc.tile_pool(name="otp", bufs=4, space="PSUM") as otp,
            ):
                # ---------- phase 0: fourier emb + scores + softmax --------
                # All 4 batches processed in single wide ops; the few
                # off-chain unary steps run on ACT to shorten the DVE chain.
                pv = posb_sb.rearrange("p (b s c) -> p b s c", b=BPC, s=2)
                x_all = pv[:, :, 0, :]   # [128, BPC, C]
                y_all = pv[:, :, 1, :]
                embq = wsb.tile([128, BPC, 4, 128], F16, tag="embq")
                for c in range(2):
                    # phase in turns: t = x*p_i + y*p_j  (>= 0, < ~19)
                    t1 = wsb.tile([128, BPC, 128], F32, tag="t1")
                    tt = wsb.tile([128, BPC, 128], F32, tag="tt")
                    nc.scalar.mul(t1, x_all, pij_sb[:, 2 * c:2 * c + 1])
                    nc.vector.tensor_scalar_mul(
                        out=tt, in0=y_all, scalar1=pij_sb[:, 2 * c + 1:2 * c + 2]
                    )
                    nc.vector.tensor_add(out=tt, in0=tt, in1=t1)
                    tc4 = wsb.tile([128, BPC, 128], F32, tag="tc4")
                    nc.vector.tensor_scalar_add(out=tc4, in0=tt, scalar1=0.25)
                    # cos chunk (t+0.25) -> q=c, sin chunk -> q=2+c.
                    # Reduce phase via round-to-nearest-even f32->i32 cast:
                    # r = t - rne(t) in [-0.5, 0.5]; sin(2pi*t) = Sin(2pi*r).
                    for src_t, q in ((tc4, c), (tt, 2 + c)):
                        ki = wsb.tile([128, BPC, 128], I32, tag="ki")
                        kf = wsb.tile([128, BPC, 128], F32, tag="kf")
                        nc.vector.tensor_copy(out=ki, in_=src_t)
                        nc.scalar.copy(out=kf, in_=ki)
                        rr = wsb.tile([128, BPC, 128], F32, tag="rr")
                        nc.vector.tensor_sub(out=rr, in0=src_t, in1=kf)
                        nc.scalar.activation(
                            out=embq[:, :, q, :], in_=rr,
                            func=mybir.ActivationFunctionType.Sin,
                            scale=TWO_PI, bias=0.0,
                        )
                # scores in fp16: 4x faster PE passes than fp32 LOW_HIGH,
                # and the Sin-table output is only ~1e-3 accurate anyway.
                scores_ps = wps.tile([O, BPC, 128], F32, tag="scores")
                for b in range(BPC):
                    for q in range(4):
                        nc.tensor.matmul(
                            out=scores_ps[:, b, :],
                            lhsT=ht4_sb[:, q * O:(q + 1) * O],
                            rhs=embq[:, b, q, :],
                            start=(q == 0), stop=(q == 3),
                        )
                # scores are bounded (|s| < ~10): plain exp is fp32-safe and
                # softmax is shift-invariant, so skip the max-subtraction.
                probs = wsb.tile([O, BPC, 128], F32, tag="probs")
                nc.scalar.activation(
                    out=probs, in_=scores_ps,
                    func=mybir.ActivationFunctionType.Exp, bias=0.0,
                )
                ssum = wsb.tile([O, BPC], F32, tag="ssum")
                nc.vector.tensor_reduce(
                    out=ssum, in_=probs, axis=mybir.AxisListType.X,
                    op=mybir.AluOpType.add,
                )
                rcp = wsb.tile([O, BPC], F32, tag="rcp")
                nc.vector.reciprocal(out=rcp, in_=ssum)
                wgt = wsb.tile([O, BPC, 128], F32, tag="wgt")
                wt_ps = wps.tile([128, BPC, O], F32, tag="wtps")
                for b in range(BPC):
                    nc.vector.tensor_scalar_mul(
                        out=wgt[:, b, :], in0=probs[:, b, :],
                        scalar1=rcp[:, b:b + 1],
                    )
                    nc.tensor.transpose(
                        out=wt_ps[:, b, :], in_=wgt[:, b, :],
                        identity=ident_sb[0:O, 0:O],
                    )
                # cast to fp16 for the streaming matmuls
                nc.vector.tensor_copy(
                    out=wt16.rearrange("p (b o) -> p b o", b=BPC), in_=wt_ps
                )

                # ---------- main loop: outT[o,t] = sum_c w[c,o]*eegT[c,t] --
                # Two batches share one 128-partition PSUM tile via column
                # tiling: batch 2p -> array cols 0-63 / PSUM partitions
                # 0-63, batch 2p+1 -> cols 64-127.  The paired matmuls run
                # concurrently on disjoint column groups, and the PSUM ->
                # SBUF cast-copy runs at full 128-lane width.
                for p in range(BPC // 2):
                    o_sb = osb.tile([128, T], F16, tag="osb")
                    for g in range(NCH):
                        ps = otp.tile([128, TCH], F32, tag="outps")
                        for h in range(2):
                            bidx = 2 * p + h
                            nc.tensor.matmul(
                                out=ps[64 * h:64 * (h + 1), :],
                                lhsT=wt16[:, bidx * O:(bidx + 1) * O],
                                rhs=e_tiles[bidx][:, g * TCH:(g + 1) * TCH],
                                start=True, stop=True,
                                tile_position=(0, 64 * h),
                            )
                        cp = nc.scalar.copy if g % 2 else nc.vector.tensor_copy
                        cp(out=o_sb[:, g * TCH:(g + 1) * TCH], in_=ps)
                        if g % QCH == QCH - 1:
                            q = g // QCH
                            hw = QCH * TCH
                            nc.sync.dma_start(
                                out=outT[2 * p:2 * p + 2, :,
                                         q * hw:(q + 1) * hw].rearrange(
                                    "b o t -> (b o) t"
                                ),
                                in_=o_sb[:, q * hw:(q + 1) * hw],
                            )
    nc.finalize()
    return nc


_NC_CACHE = None


def _get_nc():
    global _NC_CACHE
    if _NC_CACHE is None:
        _NC_CACHE = _build_nc()
    return _NC_CACHE


def _make_in_maps(eeg, positions, heads):
    pij, ht4, ident = _host_constants(np.asarray(heads, dtype=np.float32))
    eeg16 = np.asarray(eeg, dtype=np.float16)           # [B, T, C]
    positions = np.asarray(positions, dtype=np.float32)
    in_maps = []
    for core in range(N_CORES):
        sl = slice(core * BPC, (core + 1) * BPC)
        in_maps.append({
            "eegt": np.ascontiguousarray(eeg16[sl].transpose(0, 2, 1)),
            "posn": _pos_row(positions[sl]),
            "ht4": ht4,
            "pij": pij,
            "identity": ident,
        })
    return in_maps


def kernel(eeg, positions, heads, sub=None, **_unused):
    from concourse.bass_utils import run_bass_kernel_spmd

    nc = _get_nc()
    in_maps = _make_in_maps(eeg, positions, heads)
    res = run_bass_kernel_spmd(nc, in_maps, list(range(N_CORES)))
    outT = np.concatenate(
        [res.results[c]["outT"] for c in range(N_CORES)], axis=0
    )  # [B, O, T] fp16
    return outT.astype(np.float32).transpose(0, 2, 1)


# revision 13
# speedup vs baseline: 2.2945x; 1.0288x over previous
"""ChannelMerger kernel for 8x Trainium2 NeuronCores (Bass/Tile).

Computes, for eeg [B,T,C], positions [B,C,2], heads [O,D]:
    emb     = fourier_emb(positions)              # [B,C,D], D = 2*12*12
    scores  = einsum('bcd,od->boc', emb, heads)   # [B,O,C]
    weights = softmax(scores, axis=2)
    out     = einsum('bct,boc->bot', eeg_ct, weights).transpose -> [B,T,O]

Sharding: data-parallel over batch B=32 -> 4 batches per core on 8 cores.

Layout strategy: the host pre-transposes eeg to [B, C, T] and casts to
fp16, so the device streams it straight through the PE as the moving
matmul operand with the (tiny, softmaxed) channel weights stationary --
no on-device data transposes.  The result is produced as [B, O, T] fp16
(contiguous DMA) and the host casts/transposes it back to [B, T, O]
f32.  fp16 halves HBM traffic in both directions; with f32 softmax and
f32 PSUM accumulation the end-to-end relative error stays ~1e-3.

All math (fourier, scores, softmax, weighted sum) runs on-device; the
host only shards/reshapes/casts inputs and pads constants.

Weights-phase structure (it gates the first output chunk):
  - loc[ij,(b,c)] = p_i*x + p_j*y computed as ONE tiny PE matmul per
    freq chunk, contracting over a K=2 partition axis {x-row, y-row};
    this replaces a long serial DVE chain and needs no broadcast of
    positions across partitions.
  - the cos path's +0.25-turn shift rides a third contraction row of
    that matmul; range reduction keeps the proven RNE-i32-cast form
    (DVE `mod` does not exist in the ISA).
  - scores matmuls in fp16 (4x faster than fp32 LOW_HIGH passes).
Constants ride the SWDGE (gpsimd) queue so they land before the bulk
eeg stream saturates the HWDGE queue.
"""

import numpy as np

import concourse.bacc as bacc
import concourse.mybir as mybir
import concourse.tile as tile

# ---------------------------------------------------------------- constants
B, T, C = 32, 8192, 128
O = 64
N_FREQS = 12
N_IJ = N_FREQS * N_FREQS          # 144
D = 2 * N_IJ                      # 288
MARGIN = 0.2
N_CORES = 8
BPC = B // N_CORES                # batches per core = 4
TCH = 512                         # t columns per matmul chunk (1 PSUM bank)
NCH = T // TCH                    # 16
QCH = 4                           # chunks per output DMA (2048 t cols)
TH = T // 2                       # eeg half-load width
F32 = mybir.dt.float32
F16 = mybir.dt.float16


# ------------------------------------------------------------ host constants
def _host_constants(heads: np.ndarray):
    """Pure layout/padding transforms of `heads` + static tables."""
    width = 1.0 + 2.0 * MARGIN
    # Frequencies in TURNS (cycles): loc_rad = 2*pi * (pos_x*p_i + pos_y*p_j).
    p = np.arange(N_FREQS, dtype=np.float64) / width

    # lhsT rows for the loc matmul: row0 = p_i, row1 = p_j, row2 = 0.25
    # (bias row for the non-mod fallback).  Chunk c covers ij = 128c + k;
    # entries past 143 are 0 and their heads rows are zero-padded.
    p3 = np.zeros((3, 256), dtype=np.float32)
    for c in range(2):
        for k in range(128):
            ij = 128 * c + k
            if ij < N_IJ:
                p3[0, c * 128 + k] = p[ij // N_FREQS]
                p3[1, c * 128 + k] = p[ij % N_FREQS]
            p3[2, c * 128 + k] = 0.25

    # headsT chunks [K=128, O] for the 4 embT chunks (cos0, cos1, sin0, sin1)
    ht4 = np.zeros((128, 4 * O), dtype=np.float32)
    ht4[:, 0 * O:1 * O] = heads[:, 0:128].T               # cos ij 0..127
    ht4[:16, 1 * O:2 * O] = heads[:, 128:144].T           # cos ij 128..143
    ht4[:, 2 * O:3 * O] = heads[:, 144:272].T             # sin ij 0..127
    ht4[:16, 3 * O:4 * O] = heads[:, 272:288].T           # sin ij 128..143

    ident = np.eye(128, dtype=np.float32)
    return p3, ht4.astype(np.float16), ident


def _pos_rows(positions_core: np.ndarray) -> np.ndarray:
    """[BPC,C,2] -> [3, BPC*C]: row0 = x+MARGIN, row1 = y+MARGIN, row2 = 1
    (rhs rows for the loc matmul)."""
    pos = positions_core.astype(np.float32) + np.float32(MARGIN)
    r = np.ones((3, BPC * C), dtype=np.float32)
    r[0] = pos[:, :, 0].reshape(-1)
    r[1] = pos[:, :, 1].reshape(-1)
    return r


# ------------------------------------------------------------- device kernel
def _build_nc():
    # Bacc (not plain Bass): finalize() runs generate_event_semaphores,
    # which splits multi-sem waits (TRN2 allows 1 wait per instruction).
    nc = bacc.Bacc()
    eegt = nc.declare_dram_parameter("eegt", [BPC, C, T], F16, isOutput=False)
    posn = nc.declare_dram_parameter("posn", [3, BPC * C], F32, isOutput=False)
    p3 = nc.declare_dram_parameter("p3", [3, 256], F32, isOutput=False)
    ht4 = nc.declare_dram_parameter("ht4", [128, 4 * O], F16, isOutput=False)
    identity = nc.declare_dram_parameter("identity", [128, 128], F32, isOutput=False)
    outT = nc.declare_dram_parameter("outT", [BPC, O, T], F16, isOutput=True)

    TWO_PI = float(2.0 * np.pi)
    PI = float(np.pi)
    I32 = mybir.dt.int32
    BF16 = mybir.dt.bfloat16
    AF = mybir.ActivationFunctionType
    ALU = mybir.AluOpType

    with tile.TileContext(nc) as tc:
        with tc.tile_pool(name="consts", bufs=1) as cpool:
            # Weight-phase constants ride the SWDGE (gpsimd) queue: its
            # sequencer is free early and the tiny transfers land before
            # the HWDGE queue fills with bulk eeg traffic.
            posn_sb = cpool.tile([3, BPC * C], F32)
            nc.gpsimd.dma_start(out=posn_sb, in_=posn[:, :])
            p3_sb = cpool.tile([3, 256], F32)
            nc.gpsimd.dma_start(out=p3_sb, in_=p3[:, :])
            ht4_sb = cpool.tile([128, 4 * O], F16)
            nc.gpsimd.dma_start(out=ht4_sb, in_=ht4[:, :])
            ident_sb = cpool.tile([128, 128], F32)
            nc.gpsimd.dma_start(out=ident_sb, in_=identity[:, :])

            # Bulk eeg loads on the HWDGE (sync) queue, split in T-halves so
            # pair-0 matmuls can start as soon as batches 0/1 half-land.
            e_tiles = [cpool.tile([C, T], F16, name=f"e_{b}") for b in range(BPC)]
            for h in range(2):
                for b in range(BPC):
                    sl = slice(h * TH, (h + 1) * TH)
                    nc.sync.dma_start(out=e_tiles[b][:, sl], in_=eegt[b][:, sl])

            # fp16 softmaxed channel weights, transposed: [C, O] per batch
            wt16 = cpool.tile([128, BPC * O], F16)
            wu_a = cpool.tile([128, 128], BF16)
            wu_b = cpool.tile([128, 512], BF16)
            nc.vector.memset(wu_a, 1.0)
            nc.vector.memset(wu_b, 1.0)

            with (
                tc.tile_pool(name="wsb", bufs=1) as wsb,
                tc.tile_pool(name="osb", bufs=2) as osb,
            ):
                # ---------- phase 0: fourier emb ---------------------------
                # loc (in turns) via K=2 matmul; embq[:, :, q, :] holds the
                # 4 chunks (cos0, cos1, sin0, sin1) matching ht4's packing.
                embq = wsb.tile([128, BPC, 4, 128], F16, tag="embq")
                with (
                    tc.tile_pool(name="locp", bufs=4, space="PSUM") as locp,
                    tc.tile_pool(name="wups", bufs=1, space="PSUM") as wups,
                ):
                    # Per chunk, two loc matmuls: the cos path contracts a
                    # third {0.25-row x ones-row} pair so its +0.25-turn
                    # phase shift costs nothing.
                    loc_tiles = []
                    for c in range(2):
                        for rows in (3, 2):   # 3 rows -> cos (+0.25), 2 -> sin
                            loc_ps = locp.tile([128, BPC * C], F32, tag="loc")
                            nc.tensor.matmul(
                                out=loc_ps,
                                lhsT=p3_sb[0:rows, c * 128:(c + 1) * 128],
                                rhs=posn_sb[0:rows, :], start=True, stop=True,
                            )
                            loc_tiles.append(loc_ps)
                    # PE warm-up: the HAM clock gate keeps the PE at 1.2 GHz
                    # until it sees ~3.4us of sustained matmul activity; burn
                    # cheap bf16 matmuls so scores + main matmuls run at 2.4.
                    wu_ps = wups.tile([128, 512], F32)
                    for _ in range(10):
                        nc.tensor.matmul(out=wu_ps, lhsT=wu_a, rhs=wu_b,
                                         start=True, stop=True)
                    for c in range(2):
                        # q=c: cos chunk (phase +0.25 turns); q=2+c: sin.
                        for k, q in ((0, c), (1, 2 + c)):
                            lv = loc_tiles[2 * c + k].rearrange(
                                "p (b c) -> p b c", b=BPC)
                            # r = t - rne(t) in [-0.5, 0.5] via RNE i32 cast;
                            # sin(2pi*t) = Sin(2pi*r).
                            ki = wsb.tile([128, BPC, 128], I32, tag="ki")
                            kf = wsb.tile([128, BPC, 128], F32, tag="kf")
                            rr = wsb.tile([128, BPC, 128], F32, tag="rr")
                            nc.vector.tensor_copy(out=ki, in_=lv)
                            nc.vector.tensor_copy(out=kf, in_=ki)
                            nc.vector.tensor_sub(out=rr, in0=lv, in1=kf)
                            nc.scalar.activation(
                                out=embq[:, :, q, :], in_=rr,
                                func=AF.Sin, scale=TWO_PI, bias=0.0,
                            )

                # ---------- scores + softmax + main loop -------------------
                with (
                    tc.tile_pool(name="wps", bufs=1, space="PSUM") as wps,
                    tc.tile_pool(name="otp", bufs=4, space="PSUM") as otp,
                ):
                    scores_ps = wps.tile([O, BPC, 128], F32, tag="scores")
                    for b in range(BPC):
                        for q in range(4):
                            nc.tensor.matmul(
                                out=scores_ps[:, b, :],
                                lhsT=ht4_sb[:, q * O:(q + 1) * O],
                                rhs=embq[:, b, q, :],
                                start=(q == 0), stop=(q == 3),
                            )
                    # scores are bounded (|s| < ~10): plain exp is fp32-safe
                    # and softmax is shift-invariant: skip max-subtraction.
                    probs = wsb.tile([O, BPC, 128], F32, tag="probs")
                    nc.scalar.activation(out=probs, in_=scores_ps,
                                         func=AF.Exp, bias=0.0)
                    ssum = wsb.tile([O, BPC], F32, tag="ssum")
                    nc.vector.tensor_reduce(
                        out=ssum, in_=probs, axis=mybir.AxisListType.X,
                        op=ALU.add,
                    )
                    rcp = wsb.tile([O, BPC], F32, tag="rcp")
                    nc.vector.reciprocal(out=rcp, in_=ssum)
                    wgt = wsb.tile([O, BPC, 128], F32, tag="wgt")
                    wt_ps = wps.tile([128, BPC, O], F32, tag="wtps")
                    for b in range(BPC):
                        nc.vector.tensor_scalar_mul(
                            out=wgt[:, b, :], in0=probs[:, b, :],
                            scalar1=rcp[:, b:b + 1],
                        )
                        nc.tensor.transpose(
                            out=wt_ps[:, b, :], in_=wgt[:, b, :],
                            identity=ident_sb[0:O, 0:O],
                        )
                    # cast to fp16 for the streaming matmuls
                    nc.vector.tensor_copy(
                        out=wt16.rearrange("p (b o) -> p b o", b=BPC),
                        in_=wt_ps,
                    )

                    # main loop: outT[o,t] = sum_c w[c,o] * eegT[c,t].
                    # Two batches share one 128-partition PSUM tile via
                    # column tiling: batch 2p -> array cols 0-63 / PSUM
                    # partitions 0-63, batch 2p+1 -> cols 64-127.  The
                    # paired matmuls run concurrently on disjoint column
                    # groups, and the PSUM -> SBUF cast-copy runs at full
                    # 128-lane width.
                    for p in range(BPC // 2):
                        o_sb = osb.tile([128, T], F16, tag="osb")
                        for g in range(NCH):
                            ps = otp.tile([128, TCH], F32, tag="outps")
                            for h in range(2):
                                bidx = 2 * p + h
                                nc.tensor.matmul(
                                    out=ps[64 * h:64 * (h + 1), :],
                                    lhsT=wt16[:, bidx * O:(bidx + 1) * O],
                                    rhs=e_tiles[bidx][:, g * TCH:(g + 1) * TCH],
                                    start=True, stop=True,
                                    tile_position=(0, 64 * h),
                                )
                            cp = nc.scalar.copy if g % 2 else nc.vector.tensor_copy
                            cp(out=o_sb[:, g * TCH:(g + 1) * TCH], in_=ps)
                            if g % QCH == QCH - 1:
                                q = g // QCH
                                hw = QCH * TCH
                                nc.sync.dma_start(
                                    out=outT[2 * p:2 * p + 2, :,
                                             q * hw:(q + 1) * hw].rearrange(
                                        "b o t -> (b o) t"
                                    ),
                                    in_=o_sb[:, q * hw:(q + 1) * hw],
                                )
    nc.finalize()
    return nc


_NC_CACHE = None


def _get_nc():
    global _NC_CACHE
    if _NC_CACHE is None:
        _NC_CACHE = _build_nc()
    return _NC_CACHE


def _make_in_maps(eeg, positions, heads):
    p3, ht4, ident = _host_constants(np.asarray(heads, dtype=np.float32))
    eeg16 = np.asarray(eeg, dtype=np.float16)           # [B, T, C]
    positions = np.asarray(positions, dtype=np.float32)
    in_maps = []
    for core in range(N_CORES):
        sl = slice(core * BPC, (core + 1) * BPC)
        in_maps.append({
            "eegt": np.ascontiguousarray(eeg16[sl].transpose(0, 2, 1)),
            "posn": _pos_rows(positions[sl]),
            "p3": p3,
            "ht4": ht4,
            "identity": ident,
        })
    return in_maps


def kernel(eeg, positions, heads, sub=None, **_unused):
    from concourse.bass_utils import run_bass_kernel_spmd

    nc = _get_nc()
    in_maps = _make_in_maps(eeg, positions, heads)
    res = run_bass_kernel_spmd(nc, in_maps, list(range(N_CORES)))
    outT = np.concatenate(
        [res.results[c]["outT"] for c in range(N_CORES)], axis=0
    )  # [B, O, T] fp16
    return outT.astype(np.float32).transpose(0, 2, 1)


# revision 20
# speedup vs baseline: 2.3482x; 1.0234x over previous
"""ChannelMerger kernel for 8x Trainium2 NeuronCores (Bass/Tile).

Computes, for eeg [B,T,C], positions [B,C,2], heads [O,D]:
    emb     = fourier_emb(positions)              # [B,C,D], D = 2*12*12
    scores  = einsum('bcd,od->boc', emb, heads)   # [B,O,C]
    weights = softmax(scores, axis=2)
    out     = einsum('bct,boc->bot', eeg_ct, weights).transpose -> [B,T,O]

Sharding: data-parallel over batch B=32 -> 4 batches per core on 8 cores.

Layout strategy: the host pre-transposes eeg to [B, C, T] and casts to
fp16, so the device streams it straight through the PE as the moving
matmul operand with the (tiny, softmaxed) channel weights stationary --
no on-device data transposes.  The result is produced as [B, O, T] fp16
(contiguous DMA) and the host casts/transposes it back to [B, T, O]
f32.  fp16 halves HBM traffic in both directions; with f32 softmax and
f32 PSUM accumulation the end-to-end relative error stays ~1e-3.

All math (fourier, scores, softmax, weighted sum) runs on-device; the
host only shards/reshapes/casts inputs and pads constants.

Weights-phase structure (it gates the first output chunk):
  - loc[ij,(b,c)] = p_i*x + p_j*y computed as ONE tiny PE matmul per
    freq chunk, contracting over a K=2 partition axis {x-row, y-row};
    this replaces a long serial DVE chain and needs no broadcast of
    positions across partitions.
  - the cos path's +0.25-turn shift rides a third contraction row of
    that matmul; range reduction keeps the proven RNE-i32-cast form
    (DVE `mod` does not exist in the ISA).
  - scores matmuls in fp16 (4x faster than fp32 LOW_HIGH passes).
Constants ride the SWDGE (gpsimd) queue so they land before the bulk
eeg stream saturates the HWDGE queue.
"""

import numpy as np

import concourse.bacc as bacc
import concourse.mybir as mybir
import concourse.tile as tile

# ---------------------------------------------------------------- constants
B, T, C = 32, 8192, 128
O = 64
N_FREQS = 12
N_IJ = N_FREQS * N_FREQS          # 144
D = 2 * N_IJ                      # 288
MARGIN = 0.2
N_CORES = 8
BPC = B // N_CORES                # batches per core = 4
TCH = 512                         # t columns per matmul chunk (1 PSUM bank)
NCH = T // TCH                    # 16
QCH = 4                           # chunks per output DMA (2048 t cols)
TH = T // 2                       # eeg half-load width
F32 = mybir.dt.float32
F16 = mybir.dt.float16


# ------------------------------------------------------------ host constants
def _host_constants(heads: np.ndarray):
    """Pure layout/padding transforms of `heads` + static tables."""
    width = 1.0 + 2.0 * MARGIN
    # Frequencies in TURNS (cycles): loc_rad = 2*pi * (pos_x*p_i + pos_y*p_j).
    p = np.arange(N_FREQS, dtype=np.float64) / width

    # lhsT rows for the fp16 loc matmul.  fp32 PE matmuls run 2x LOW_HIGH
    # passes, so split p into fp16 hi+lo instead: rows pair with the posn
    # rows as (p_hi,x_hi) (q_hi,y_hi) (p_hi,x_lo) (q_hi,y_lo) (p_lo,x_hi)
    # (q_lo,y_hi) (0.25,1).  fp16 products accumulate exactly in f32; the
    # dropped lo*lo terms are ~1e-5 turns.  Chunk c covers ij = 128c + k;
    # entries past 143 are 0 and their heads rows are zero-padded.
    pi = np.zeros((1, 256))
    pj = np.zeros((1, 256))
    for c in range(2):
        for k in range(128):
            ij = 128 * c + k
            if ij < N_IJ:
                pi[0, c * 128 + k] = p[ij // N_FREQS]
                pj[0, c * 128 + k] = p[ij % N_FREQS]
    pi_hi = pi.astype(np.float16).astype(np.float64)
    pj_hi = pj.astype(np.float16).astype(np.float64)
    pi_lo = (pi - pi_hi).astype(np.float16).astype(np.float64)
    pj_lo = (pj - pj_hi).astype(np.float16).astype(np.float64)
    p7 = np.concatenate(
        [pi_hi, pj_hi, pi_hi, pj_hi, pi_lo, pj_lo,
         np.full((1, 256), 0.25)], axis=0,
    ).astype(np.float16)

    # headsT chunks [K=128, O] for the 4 embT chunks (cos0, cos1, sin0, sin1)
    ht4 = np.zeros((128, 4 * O), dtype=np.float32)
    ht4[:, 0 * O:1 * O] = heads[:, 0:128].T               # cos ij 0..127
    ht4[:16, 1 * O:2 * O] = heads[:, 128:144].T           # cos ij 128..143
    ht4[:, 2 * O:3 * O] = heads[:, 144:272].T             # sin ij 0..127
    ht4[:16, 3 * O:4 * O] = heads[:, 272:288].T           # sin ij 128..143

    ident = np.eye(128, dtype=np.float32)
    return p7, ht4.astype(np.float16), ident


def _pos_rows(positions_core: np.ndarray) -> np.ndarray:
    """[BPC,C,2] -> fp16 [7, BPC*C] rhs rows for the loc matmul:
    x_hi, y_hi, x_lo, y_lo, x_hi, y_hi, 1 (hi/lo fp16 split of pos+MARGIN,
    matching the p7 row pairing)."""
    pos = positions_core.astype(np.float64) + MARGIN
    x, y = pos[:, :, 0].reshape(-1), pos[:, :, 1].reshape(-1)
    x_hi = x.astype(np.float16).astype(np.float64)
    y_hi = y.astype(np.float16).astype(np.float64)
    x_lo = x - x_hi
    y_lo = y - y_hi
    r = np.ones((7, BPC * C), dtype=np.float16)
    r[0], r[1] = x_hi, y_hi
    r[2], r[3] = x_lo, y_lo
    r[4], r[5] = x_hi, y_hi
    return r


# ------------------------------------------------------------- device kernel
def _build_nc():
    # Bacc (not plain Bass): finalize() runs generate_event_semaphores,
    # which splits multi-sem waits (TRN2 allows 1 wait per instruction).
    nc = bacc.Bacc()
    eegt = nc.declare_dram_parameter("eegt", [BPC, C, T], F16, isOutput=False)
    posn = nc.declare_dram_parameter("posn", [7, BPC * C], F16, isOutput=False)
    p7 = nc.declare_dram_parameter("p7", [7, 256], F16, isOutput=False)
    ht4 = nc.declare_dram_parameter("ht4", [128, 4 * O], F16, isOutput=False)
    identity = nc.declare_dram_parameter("identity", [128, 128], F32, isOutput=False)
    outT = nc.declare_dram_parameter("outT", [BPC, O, T], F16, isOutput=True)

    TWO_PI = float(2.0 * np.pi)
    PI = float(np.pi)
    I32 = mybir.dt.int32
    BF16 = mybir.dt.bfloat16
    AF = mybir.ActivationFunctionType
    ALU = mybir.AluOpType

    with tile.TileContext(nc) as tc:
        with tc.tile_pool(name="consts", bufs=1) as cpool:
            # Weight-phase constants ride the ACT (scalar) HWDGE queue: it
            # is free this early, so the tiny transfers issue immediately
            # without delaying the bulk eeg issues on the sync queue.
            # (SWDGE/gpsimd was tried and is worse: ~1us Q7 descriptor
            # generation each and packets starve behind the eeg stream.)
            posn_sb = cpool.tile([7, BPC * C], F16)
            nc.scalar.dma_start(out=posn_sb, in_=posn[:, :])
            p7_sb = cpool.tile([7, 256], F16)
            nc.scalar.dma_start(out=p7_sb, in_=p7[:, :])
            ht4_sb = cpool.tile([128, 4 * O], F16)
            nc.scalar.dma_start(out=ht4_sb, in_=ht4[:, :])
            ident_sb = cpool.tile([128, 128], F32)
            nc.scalar.dma_start(out=ident_sb, in_=identity[:, :])

            # Bulk eeg loads on the HWDGE (sync) queue, split in T-halves so
            # pair-0 matmuls can start as soon as batches 0/1 half-land.
            e_tiles = [cpool.tile([C, T], F16, name=f"e_{b}") for b in range(BPC)]
            for h in range(2):
                for b in range(BPC):
                    sl = slice(h * TH, (h + 1) * TH)
                    nc.sync.dma_start(out=e_tiles[b][:, sl], in_=eegt[b][:, sl])

            # fp16 softmaxed channel weights, transposed: [C, O] per batch
            wt16 = cpool.tile([128, BPC * O], F16)
            wu_a = cpool.tile([128, 128], BF16)
            wu_b = cpool.tile([128, 512], BF16)
            nc.vector.memset(wu_a, 1.0)
            nc.vector.memset(wu_b, 1.0)

            with (
                tc.tile_pool(name="wsb", bufs=1) as wsb,
                tc.tile_pool(name="osb", bufs=2) as osb,
            ):
                # ---------- phase 0: fourier emb ---------------------------
                # loc (in turns) via K=2 matmul; embq[:, :, q, :] holds the
                # 4 chunks (cos0, cos1, sin0, sin1) matching ht4's packing.
                embq = wsb.tile([128, BPC, 4, 128], F16, tag="embq")
                with (
                    tc.tile_pool(name="locp", bufs=4, space="PSUM") as locp,
                    tc.tile_pool(name="wups", bufs=1, space="PSUM") as wups,
                ):
                    # PE warm-up FIRST in the PE queue: the HAM clock gate
                    # keeps the PE at 1.2 GHz until it sees ~3.4us of
                    # sustained matmul activity; burn cheap bf16 matmuls
                    # (no data deps) so everything after runs at 2.4 GHz.
                    wu_ps = wups.tile([128, 512], F32)
                    for _ in range(12):
                        nc.tensor.matmul(out=wu_ps, lhsT=wu_a, rhs=wu_b,
                                         start=True, stop=True)
                    # Per chunk, two fp16 loc matmuls: sin path contracts
                    # the 6 hi/lo split rows, cos path adds the
                    # {0.25-row x ones-row} pair so its +0.25-turn phase
                    # shift costs nothing.
                    loc_tiles = []
                    for c in range(2):
                        for rows in (7, 6):   # 7 rows -> cos (+0.25), 6 -> sin
                            loc_ps = locp.tile([128, BPC * C], F32, tag="loc")
                            nc.tensor.matmul(
                                out=loc_ps,
                                lhsT=p7_sb[0:rows, c * 128:(c + 1) * 128],
                                rhs=posn_sb[0:rows, :], start=True, stop=True,
                            )
                            loc_tiles.append(loc_ps)
                    for c in range(2):
                        # q=c: cos chunk (phase +0.25 turns); q=2+c: sin.
                        for k, q in ((0, c), (1, 2 + c)):
                            lv = loc_tiles[2 * c + k].rearrange(
                                "p (b c) -> p b c", b=BPC)
                            # r = t - rne(t) in [-0.5, 0.5] via RNE i32 cast;
                            # sin(2pi*t) = Sin(2pi*r).
                            ki = wsb.tile([128, BPC, 128], I32, tag="ki")
                            kf = wsb.tile([128, BPC, 128], F32, tag="kf")
                            rr = wsb.tile([128, BPC, 128], F32, tag="rr")
                            nc.vector.tensor_copy(out=ki, in_=lv)
                            nc.vector.tensor_copy(out=kf, in_=ki)
                            nc.vector.tensor_sub(out=rr, in0=lv, in1=kf)
                            nc.scalar.activation(
                                out=embq[:, :, q, :], in_=rr,
                                func=AF.Sin, scale=TWO_PI, bias=0.0,
                            )

                # ---------- scores + softmax + main loop -------------------
                with (
                    tc.tile_pool(name="wps", bufs=1, space="PSUM") as wps,
                    tc.tile_pool(name="otp", bufs=4, space="PSUM") as otp,
                ):
                    scores_ps = wps.tile([O, BPC, 128], F32, tag="scores")
                    for b in range(BPC):
                        for q in range(4):
                            nc.tensor.matmul(
                                out=scores_ps[:, b, :],
                                lhsT=ht4_sb[:, q * O:(q + 1) * O],
                                rhs=embq[:, b, q, :],
                                start=(q == 0), stop=(q == 3),
                            )
                    # scores are bounded (|s| < ~10): plain exp is fp32-safe
                    # and softmax is shift-invariant: skip max-subtraction.
                    probs = wsb.tile([O, BPC, 128], F32, tag="probs")
                    nc.scalar.activation(out=probs, in_=scores_ps,
                                         func=AF.Exp, bias=0.0)
                    ssum = wsb.tile([O, BPC], F32, tag="ssum")
                    nc.vector.tensor_reduce(
                        out=ssum, in_=probs, axis=mybir.AxisListType.X,
                        op=ALU.add,
                    )
                    rcp = wsb.tile([O, BPC], F32, tag="rcp")
                    nc.vector.reciprocal(out=rcp, in_=ssum)
                    wgt = wsb.tile([O, BPC, 128], F32, tag="wgt")
                    wt_ps = wps.tile([128, BPC, O], F32, tag="wtps")
                    for b in range(BPC):
                        nc.vector.tensor_scalar_mul(
                            out=wgt[:, b, :], in0=probs[:, b, :],
                            scalar1=rcp[:, b:b + 1],
                        )
                        nc.tensor.transpose(
                            out=wt_ps[:, b, :], in_=wgt[:, b, :],
                            identity=ident_sb[0:O, 0:O],
                        )
                    # cast to fp16 for the streaming matmuls
                    nc.vector.tensor_copy(
                        out=wt16.rearrange("p (b o) -> p b o", b=BPC),
                        in_=wt_ps,
                    )

                    # main loop: outT[o,t] = sum_c w[c,o] * eegT[c,t].
                    # Two batches share one 128-partition PSUM tile via
                    # column tiling: batch 2p -> array cols 0-63 / PSUM
                    # partitions 0-63, batch 2p+1 -> cols 64-127.  The
                    # paired matmuls run concurrently on disjoint column
                    # groups, and the PSUM -> SBUF cast-copy runs at full
                    # 128-lane width.
                    for p in range(BPC // 2):
                        o_sb = osb.tile([128, T], F16, tag="osb")
                        for g in range(NCH):
                            ps = otp.tile([128, TCH], F32, tag="outps")
                            for h in range(2):
                                bidx = 2 * p + h
                                nc.tensor.matmul(
                                    out=ps[64 * h:64 * (h + 1), :],
                                    lhsT=wt16[:, bidx * O:(bidx + 1) * O],
                                    rhs=e_tiles[bidx][:, g * TCH:(g + 1) * TCH],
                                    start=True, stop=True,
                                    tile_position=(0, 64 * h),
                                )
                            cp = nc.scalar.copy if g % 2 else nc.vector.tensor_copy
                            cp(out=o_sb[:, g * TCH:(g + 1) * TCH], in_=ps)
                            if g % QCH == QCH - 1:
                                q = g // QCH
                                hw = QCH * TCH
                                nc.sync.dma_start(
                                    out=outT[2 * p:2 * p + 2, :,
                                             q * hw:(q + 1) * hw].rearrange(
                                        "b o t -> (b o) t"
                                    ),
                                    in_=o_sb[:, q * hw:(q + 1) * hw],
                                )
    nc.finalize()
    return nc


_NC_CACHE = None


def _get_nc():
    global _NC_CACHE
    if _NC_CACHE is None:
        _NC_CACHE = _build_nc()
    return _NC_CACHE


def _make_in_maps(eeg, positions, heads):
    p7, ht4, ident = _host_constants(np.asarray(heads, dtype=np.float32))
    eeg16 = np.asarray(eeg, dtype=np.float16)           # [B, T, C]
    positions = np.asarray(positions, dtype=np.float32)
    in_maps = []
    for core in range(N_CORES):
        sl = slice(core * BPC, (core + 1) * BPC)
        in_maps.append({
            "eegt": np.ascontiguousarray(eeg16[sl].transpose(0, 2, 1)),
            "posn": _pos_rows(positions[sl]),
            "p7": p7,
            "ht4": ht4,
            "identity": ident,
        })
    return in_maps


def kernel(eeg, positions, heads, sub=None, **_unused):
    from concourse.bass_utils import run_bass_kernel_spmd

    nc = _get_nc()
    in_maps = _make_in_maps(eeg, positions, heads)
    res = run_bass_kernel_spmd(nc, in_maps, list(range(N_CORES)))
    outT = np.concatenate(
        [res.results[c]["outT"] for c in range(N_CORES)], axis=0
    )  # [B, O, T] fp16
    return outT.astype(np.float32).transpose(0, 2, 1)
